# revision 13
# baseline (speedup 1.0000x reference)
"""NF4-style 4-bit quantized linear: out = x @ dequant(w).T on 8 TRN2 NeuronCores.

Column-parallel sharding: core c owns output features [c*512, (c+1)*512) and the
matching slices of the packed weight + quant state arrays; x is replicated.

Per core:
  1. dequantize the 512x4096 weight slice on-chip (DVE, u8/f16 ops) in
     progressive k-chunks, round-tripping each chunk through DRAM with an xbar
     transpose to [k-partition, outf] layout,
  2. stream x through xbar transpose DMAs ([token, k] -> [k, token]) with two
     small 128-token lead blocks, and run the fp16 matmul on the PE array,
     accumulating in PSUM over 32 k-tiles.

Queue discipline (critical for the startup ramp):
  - ALL xbar transposes ride the ACT (scalar) HWDGE ring: concurrent
    transposes on the two HWDGE rings corrupt data (shared xbar), so they
    must be on one ring. Emission order: xtr0, xtr1, all W chunks, rest.
  - ALL plain DMAs (packed-scale load, packed-weight load, w-chunk stores,
    output writes) ride the SP (sync) HWDGE ring; plain HWDGE DMAs overlap
    in-flight transposes, unlike SWDGE (gpsimd) DMAs which Tile serializes
    against them. No gpsimd DMAs anywhere.
Host packs the five quant-state arrays into one f32 tensor (one DMA) and
provides qw as uint8; host gathers per-core outputs by concat along axis 1.
"""
import numpy as np

import concourse.bass as bass
import concourse.mybir as mybir
import concourse.tile as tile
from concourse import bacc
from concourse.tile_rust import add_dep_helper as tile_rust_add_dep
from concourse.bass_utils import run_bass_kernel_spmd

F16 = mybir.dt.float16
F32 = mybir.dt.float32
U8 = mybir.dt.uint8
Alu = mybir.AluOpType

P = 128
TOKENS = 8192
IN_F = 4096
OUT_F = 4096
N_CORES = 8
O_C = OUT_F // N_CORES          # 512 out features per core
KT = IN_F // P                  # 32 k-tiles
BPR = IN_F // 2                 # 2048 packed bytes per weight row
NB_O = O_C // P                 # 4 o-tiles of 128 rows

CHUNKS = [2, 2, 4, 8, 8, 8]    # k-tiles per dequant chunk (progressive)
KOFF = [0, 2, 4, 8, 16, 24]    # k-tile offset of each chunk
X_BLOCKS = [256, 256, 512] + [512] * 14   # token blocks
N_RAMP = 3


def _build(tokens=TOKENS):
    nc = bacc.Bacc("TRN2", target_bir_lowering=False, debug=False,
                   enable_asserts=False)

    x = nc.dram_tensor("x", [tokens, IN_F], F16, kind="ExternalInput").ap()
    qw = nc.dram_tensor("qw", [O_C, BPR], U8, kind="ExternalInput").ap()
    # packed quant state: [qam | qcode | qoff | am2 | c2] along columns
    scp = nc.dram_tensor("scp", [O_C, 224], F32, kind="ExternalInput").ap()
    out = nc.dram_tensor("out", [tokens, O_C], F16, kind="ExternalOutput").ap()

    kk2chunk = []
    for ci, c in enumerate(CHUNKS):
        for j in range(c):
            kk2chunk.append((ci, j))

    with tile.TileContext(nc) as tc:
        with tc.tile_pool(name="wt_pool", bufs=1) as wt_pool, \
             tc.tile_pool(name="wdram", bufs=1, space="DRAM") as wdram, \
             tc.tile_pool(name="sc_pool", bufs=1) as sc_pool, \
             tc.tile_pool(name="dq", bufs=1) as dq, \
             tc.tile_pool(name="xt_pool", bufs=2) as xt_pool, \
             tc.tile_pool(name="ps_pool", bufs=8, space="PSUM") as ps_pool, \
             tc.tile_pool(name="ob_pool", bufs=2) as ob_pool:
            wts = [wt_pool.tile([P, c, O_C], F16, name=f"wt{ci}")
                   for ci, c in enumerate(CHUNKS)]
            wds = [wdram.tile([O_C, c * P], F16, name=f"wd{ci}")
                   for ci, c in enumerate(CHUNKS)]

            # Effectively all DMA traffic serializes against in-flight xbar
            # transposes, so pin ONE explicit global order tuned for the ramp:
            # loads, xtr0, xtr1, (store_i, wt_i)*, xtr2, steady xts.
            chain = []

            # ---- plain loads first (SP ring), then the two ramp x blocks
            qt_all = dq.tile([P, NB_O, BPR], U8, name="qt_all")
            chain.append(nc.sync.dma_start(
                qt_all, qw.rearrange("(a p) c -> p a c", p=P)))
            sc3 = sc_pool.tile([P, NB_O, 224], F32, name="sc3")
            chain.append(nc.sync.dma_start(
                sc3, scp.rearrange("(a p) c -> p a c", p=P)))

            xtiles, xinsts = [], []
            r0 = 0
            for bi in range(3):
                t = xt_pool.tile([P, KT, X_BLOCKS[bi]], F16,
                                 name=f"xtr{bi}", bufs=1)
                xi = nc.scalar.dma_start(out=t, in_=x[r0:r0 + X_BLOCKS[bi], :],
                                         transpose=True)
                xinsts.append(xi)
                chain.append(xi)
                xtiles.append(t)
                r0 += X_BLOCKS[bi]

            # ---- scale math on DVE -> f16 scale/offset tiles
            am3 = sc3[:, :, 0:64]
            cd3 = sc3[:, :, 64:128]
            of3 = sc3[:, :, 128:192]
            am23 = sc3[:, :, 192:208]
            c23 = sc3[:, :, 208:224]
            rc = sc_pool.tile([P, NB_O, 64], F32, name="rc")
            nc.vector.reciprocal(rc, cd3)
            s1 = sc_pool.tile([P, NB_O, 64], F32, name="s1")
            nc.vector.tensor_tensor(s1, am3, rc, Alu.mult)
            rc2 = sc_pool.tile([P, NB_O, 16], F32, name="rc2")
            nc.vector.reciprocal(rc2, c23)
            s2 = sc_pool.tile([P, NB_O, 16], F32, name="s2")
            nc.vector.tensor_tensor(s2, am23, rc2, Alu.mult)
            S3 = sc_pool.tile([P, NB_O, 64], F32, name="S3")
            nc.vector.tensor_tensor(
                S3, s1, s2.unsqueeze(3).broadcast_to([P, NB_O, 16, 4]), Alu.mult)
            offS3 = sc_pool.tile([P, NB_O, 64], F32, name="offS3")
            nc.vector.tensor_tensor(offS3, of3, S3, Alu.mult)
            S3h = sc_pool.tile([P, NB_O, 64], F16, name="S3h")
            nc.vector.tensor_copy(S3h, S3)
            offS3h = sc_pool.tile([P, NB_O, 64], F16, name="offS3h")
            nc.vector.tensor_copy(offS3h, offS3)

            # ---- dequant chunks: DVE -> store (SP) -> transpose (ACT)
            for ci, c in enumerate(CHUNKS):
                bcc = 64 * c
                nbc = 2 * c
                b0 = KOFF[ci] * 64
                qt = qt_all[:, :, b0:b0 + bcc]
                hi = dq.tile([P, NB_O, bcc], U8, name="hi")
                nc.vector.tensor_scalar(hi, qt, 4, None,
                                        Alu.logical_shift_right)
                lo = dq.tile([P, NB_O, bcc], U8, name="lo")
                nc.vector.tensor_scalar(lo, qt, 15, None, Alu.bitwise_and)
                bsl = slice(KOFF[ci] * 2, KOFF[ci] * 2 + nbc)
                S_b = S3h[:, :, bsl].unsqueeze(3) \
                    .broadcast_to([P, NB_O, nbc, 32])
                offS_b = offS3h[:, :, bsl].unsqueeze(3) \
                    .broadcast_to([P, NB_O, nbc, 32])
                we = dq.tile([P, NB_O, bcc], F16, name="we")
                nc.vector.tensor_tensor(we, lo, S_b, Alu.mult)
                wo = dq.tile([P, NB_O, bcc], F16, name="wo")
                nc.vector.tensor_tensor(wo, hi, S_b, Alu.mult)
                w_nat = dq.tile([P, NB_O, 128 * c], F16, name="w_nat", bufs=2)
                nc.vector.tensor_tensor(
                    w_nat[:, :, 0::2], we, offS_b, Alu.subtract)
                nc.vector.tensor_tensor(
                    w_nat[:, :, 1::2], wo, offS_b, Alu.subtract)
                chain.append(nc.sync.dma_start(
                    wds[ci].rearrange("(a p) c -> p a c", p=P), w_nat))
                chain.append(nc.scalar.dma_start(out=wts[ci],
                                                 in_=wds[ci][:, :],
                                                 transpose=True))

            def evac(ps, row0):
                ob = ob_pool.tile([P, O_C], F16, name="ob")
                nc.vector.tensor_copy(ob, ps)
                nc.sync.dma_start(out[row0:row0 + P, :], ob)

            # ramp wave: blocks 0,1,2 (256+256+512 tokens = all 8 PSUM
            # banks) interleaved kk-outer -- 8 MMs per arriving k-tile
            # (55us of PE work) fully absorb the W-chunk trickle window.
            ramp = [(0, 0), (0, 1), (1, 0), (1, 1),
                    (2, 0), (2, 1), (2, 2), (2, 3)]
            psA = [ps_pool.tile([P, O_C], F32, name="ps") for _ in range(8)]
            for kk in range(KT):
                ci, j = kk2chunk[kk]
                for i, (b, st) in enumerate(ramp):
                    nc.tensor.matmul(
                        psA[i], xtiles[b][:, kk, st * P:(st + 1) * P],
                        wts[ci][:, j, :],
                        start=(kk == 0), stop=(kk == KT - 1))
            for i in range(8):
                evac(psA[i], i * P)

            # steady blocks: st-outer, kk-inner (W fully resident)
            row0 = 1024
            for bi in range(N_RAMP, len(X_BLOCKS)):
                tb = X_BLOCKS[bi]
                xt = xt_pool.tile([P, KT, tb], F16, name="xt")
                xi = nc.scalar.dma_start(
                    out=xt, in_=x[r0:r0 + tb, :], transpose=True)
                xinsts.append(xi)
                chain.append(xi)
                r0 += tb
                for st in range(tb // P):
                    ps = ps_pool.tile([P, O_C], F32, name="ps")
                    for kk in range(KT):
                        ci, j = kk2chunk[kk]
                        nc.tensor.matmul(
                            ps, xt[:, kk, st * P:(st + 1) * P],
                            wts[ci][:, j, :],
                            start=(kk == 0), stop=(kk == KT - 1))
                    evac(ps, row0)
                    row0 += P

            for a, b in zip(chain[1:], chain):
                tile_rust_add_dep(a.ins, b.ins, True, "global dma order")

    nc.compile()
    return nc


_NC_CACHE = {}


def _get_nc(tokens=TOKENS):
    if tokens not in _NC_CACHE:
        _NC_CACHE[tokens] = _build(tokens)
    return _NC_CACHE[tokens]


def _shard(inputs):
    x = np.ascontiguousarray(np.asarray(inputs["x"], dtype=np.float16))
    qw = np.asarray(inputs["quantized_weight"], dtype=np.int32).astype(np.uint8)
    qam = np.asarray(inputs["quant_absmax"], dtype=np.int32).astype(np.float32)
    qcode = np.asarray(inputs["quant_code"], dtype=np.float32)
    qoff = np.asarray(inputs["quant_offset"], dtype=np.float32)
    am2 = np.asarray(inputs["state2_absmax"], dtype=np.float32)
    c2 = np.asarray(inputs["state2_code"], dtype=np.float32)

    pb = O_C * BPR        # packed bytes per core
    nb1 = O_C * 64        # primary blocks per core
    nb2 = O_C * 16        # secondary blocks per core
    in_maps = []
    for c in range(N_CORES):
        scp = np.concatenate([
            qam[c * nb1:(c + 1) * nb1].reshape(O_C, 64),
            qcode[c * nb1:(c + 1) * nb1].reshape(O_C, 64),
            qoff[c * nb1:(c + 1) * nb1].reshape(O_C, 64),
            am2[c * nb2:(c + 1) * nb2].reshape(O_C, 16),
            c2[c * nb2:(c + 1) * nb2].reshape(O_C, 16),
        ], axis=1)
        in_maps.append({
            "x": x,
            "qw": np.ascontiguousarray(
                qw[c * pb:(c + 1) * pb].reshape(O_C, BPR)),
            "scp": np.ascontiguousarray(scp),
        })
    return in_maps


def _run(inputs, trace=False, trace_cores=None):
    nc = _get_nc()
    in_maps = _shard(inputs)
    res = run_bass_kernel_spmd(
        nc, in_maps, list(range(N_CORES)), trace=trace,
        trace_cores=trace_cores)
    out = np.concatenate([r["out"] for r in res.results], axis=1)
    return out, res


def kernel(**inputs) -> np.ndarray:
    out, _ = _run(inputs, trace=False)
    return out
